# revision 4
# baseline (speedup 1.0000x reference)
"""GCN message-passing kernel for trn2, 8-core SPMD.

Sharding: nodes (targets) split into 8 shards of 12544 (100352 padded).
Edges partitioned by target shard. Per layer: transformed+scaled node
features (z = dinv * (x @ W)) are AllGathered into a bf16 table
(128-wide rows = 256B, dma_gather's granularity); each core gathers its
edges' source rows in 4 source-chunk passes (int16 index reach), then
scatter-reduces onto its 12544 targets with one-hot matmuls on the PE
(S tiles precomputed host-side), accumulating in PSUM + an SBUF
accumulator. Final Dense head fused on-device.
"""

import os
import numpy as np
import ml_dtypes

import concourse.bacc as bacc
import concourse.mybir as mybir
import concourse.tile as tile
from concourse import bass_utils

N = 100000
NV = 100352          # padded node count (8 * 12544)
SH = 12544           # targets per core
SR = 12545           # allgather shard rows (+1 zero row)
TR = 8 * SR          # table rows
NTB = SH // 128      # 98 target blocks per core
NCHUNK = 4           # source chunks (int16 reach)
CS = 2 * SH          # nodes per chunk
CSR = 2 * SR         # table rows per chunk
NCORE = 8
NI = 8192            # gather indices per call (desc ring limit)
F0, F1, F2, F3, FC = 128, 64, 32, 16, 16

FP32 = mybir.dt.float32
BF16 = mybir.dt.bfloat16
I16 = mybir.dt.int16

LAST_EXEC_NS = None


def _host_prep(edges):
    """Build per-core slot layout, gather indices, one-hot S tiles."""
    e0 = np.asarray(edges[0], dtype=np.int64)
    e1 = np.asarray(edges[1], dtype=np.int64)
    row = np.concatenate([e0, np.arange(N, dtype=np.int64)])
    col = np.concatenate([e1, np.arange(N, dtype=np.int64)])
    deg = np.bincount(col, minlength=N).astype(np.float64)
    dinv = (1.0 / np.sqrt(np.maximum(deg, 1.0))).astype(np.float32)
    dinv_pad = np.ones(NV, np.float32)
    dinv_pad[:N] = dinv

    core = col // SH
    per_core = []
    counts = np.zeros((NCORE, NCHUNK, NTB), np.int64)
    for c in range(NCORE):
        m = core == c
        r, t = row[m], col[m] - SH * c
        ch = r // CS
        tb = t // 128
        order = np.lexsort((t, tb, ch))
        r, t, ch, tb = r[order], t[order], ch[order], tb[order]
        key = ch * NTB + tb
        counts[c] += np.bincount(key, minlength=NCHUNK * NTB).reshape(
            NCHUNK, NTB
        )
        per_core.append((r, t, key))

    padded = (np.ceil(counts.max(axis=0) / 128.0) * 128).astype(np.int64)
    group_base = np.concatenate([[0], np.cumsum(padded.reshape(-1))[:-1]])
    s_tot = int(padded.sum())
    n_tiles = s_tot // 128
    chunk_slots = padded.sum(axis=1)          # slots per chunk
    chunk_base = np.concatenate([[0], np.cumsum(chunk_slots)[:-1]])

    # tile -> target block map (same for all cores)
    tile_tb = np.repeat(
        np.tile(np.arange(NTB), NCHUNK), (padded.reshape(-1) // 128)
    )
    tile_chunk = np.repeat(np.arange(NCHUNK), (padded.sum(axis=1) // 128))

    gidx_cores, st_cores = [], []
    for c in range(NCORE):
        r, t, key = per_core[c]
        sizes = np.bincount(key, minlength=NCHUNK * NTB)
        first = np.concatenate([[0], np.cumsum(sizes)[:-1]])
        rank = np.arange(len(r)) - np.repeat(first, sizes)
        pos = group_base[key] + rank
        shard = r // SH
        loc = r % SH
        idx_in_chunk = (shard % 2) * SR + loc

        slot_idx = np.zeros(s_tot, np.int16)
        slot_idx[pos] = idx_in_chunk.astype(np.int16)
        slot_col = np.full(s_tot, -1, np.int32)
        slot_col[pos] = (t % 128).astype(np.int32)

        # wrapped [16, s_tot/16] -> replicate to 128 partitions
        w = slot_idx.reshape(-1, 16).T
        gidx_cores.append(np.tile(w, (8, 1)).astype(np.int16))

        st = np.zeros((128, n_tiles * 128), np.uint16)
        i = np.flatnonzero(slot_col >= 0)
        p = i % 128
        j = i // 128
        st[p, j * 128 + slot_col[i]] = 0x3F80  # bf16 1.0
        st_cores.append(st.view(ml_dtypes.bfloat16))

    # gather calls: per chunk, slices of <= NI slots
    calls = []
    for ch in range(NCHUNK):
        base, tot = int(chunk_base[ch]), int(chunk_slots[ch])
        off = 0
        while off < tot:
            n = min(NI, tot - off)
            calls.append((ch, base + off, n))
            off += n

    meta = dict(
        n_tiles=n_tiles, tile_tb=tile_tb, tile_chunk=tile_chunk,
        calls=calls, s_tot=s_tot,
    )
    return meta, gidx_cores, st_cores, dinv_pad


def _build(meta):
    nc = bacc.Bacc(None)
    xt = nc.declare_dram_parameter("xt", [128, SH], FP32, isOutput=False)
    gidx = nc.declare_dram_parameter(
        "gidx", [128, meta["s_tot"] // 16], I16, isOutput=False)
    st = nc.declare_dram_parameter(
        "st", [128, meta["n_tiles"] * 128], BF16, isOutput=False)
    dinv_t = nc.declare_dram_parameter("dinv_t", [128, NTB], FP32, isOutput=False)
    w1 = nc.declare_dram_parameter("w1", [128, F1], FP32, isOutput=False)
    w2 = nc.declare_dram_parameter("w2", [F1, F2], FP32, isOutput=False)
    w3 = nc.declare_dram_parameter("w3", [F2, F3], FP32, isOutput=False)
    wf1 = nc.declare_dram_parameter("wf1", [F1, FC], FP32, isOutput=False)
    wf2 = nc.declare_dram_parameter("wf2", [F2, FC], FP32, isOutput=False)
    wf3 = nc.declare_dram_parameter("wf3", [F3, FC], FP32, isOutput=False)
    b1 = nc.declare_dram_parameter("b1", [128, F1], FP32, isOutput=False)
    b2 = nc.declare_dram_parameter("b2", [128, F2], FP32, isOutput=False)
    b3 = nc.declare_dram_parameter("b3", [128, F3], FP32, isOutput=False)
    bf = nc.declare_dram_parameter("bf", [128, FC], FP32, isOutput=False)
    ident = nc.declare_dram_parameter("ident", [128, 128], FP32, isOutput=False)
    out = nc.declare_dram_parameter("out", [SH, FC], FP32, isOutput=True)

    tile_tb, tile_chunk = meta["tile_tb"], meta["tile_chunk"]
    calls, n_tiles = meta["calls"], meta["n_tiles"]
    widths = [F1, F2, F3]

    with tile.TileContext(nc) as tc:
        with (
            tc.tile_pool(name="const", bufs=1) as cpool,
            tc.tile_pool(name="slab", bufs=2) as spool,
            tc.tile_pool(name="stp", bufs=2) as stpool,
            tc.tile_pool(name="accp", bufs=1) as apool,
            tc.tile_pool(name="fpool", bufs=1) as fpool,
            tc.tile_pool(name="scr", bufs=2) as scr,
            tc.tile_pool(name="xtp", bufs=2) as xtp,
            tc.tile_pool(name="idxp", bufs=2) as idxp,
            tc.tile_pool(name="ps", bufs=4, space="PSUM") as psum,
            tc.tile_pool(name="ptr", bufs=2, space="PSUM") as ptrans,
            tc.tile_pool(name="pz", bufs=2, space="PSUM") as pzp,
            tc.tile_pool(name="dram", bufs=1, space="DRAM") as dram,
        ):
            dinv_sb = cpool.tile([128, NTB], FP32)
            nc.sync.dma_start(dinv_sb[:], dinv_t[:])
            w1_sb = cpool.tile([128, F1], FP32)
            nc.sync.dma_start(w1_sb[:], w1[:])
            w2_sb = cpool.tile([F1, F2], FP32)
            nc.sync.dma_start(w2_sb[:], w2[:])
            w3_sb = cpool.tile([F2, F3], FP32)
            nc.sync.dma_start(w3_sb[:], w3[:])
            wf1_sb = cpool.tile([F1, FC], FP32)
            nc.sync.dma_start(wf1_sb[:], wf1[:])
            wf2_sb = cpool.tile([F2, FC], FP32)
            nc.sync.dma_start(wf2_sb[:], wf2[:])
            wf3_sb = cpool.tile([F3, FC], FP32)
            nc.sync.dma_start(wf3_sb[:], wf3[:])
            b1_sb = cpool.tile([128, F1], FP32)
            nc.sync.dma_start(b1_sb[:], b1[:])
            b2_sb = cpool.tile([128, F2], FP32)
            nc.sync.dma_start(b2_sb[:], b2[:])
            b3_sb = cpool.tile([128, F3], FP32)
            nc.sync.dma_start(b3_sb[:], b3[:])
            bf_sb = cpool.tile([128, FC], FP32)
            nc.sync.dma_start(bf_sb[:], bf[:])
            id_sb = cpool.tile([128, 128], FP32)
            nc.sync.dma_start(id_sb[:], ident[:])
            zero_sb = cpool.tile([128, 128], BF16)
            nc.gpsimd.memset(zero_sb[:], 0.0)

            acc = apool.tile([128, NTB * F1], FP32)
            f1_sb = fpool.tile([128, NTB * F1], FP32)
            f2_sb = fpool.tile([128, NTB * F2], FP32)
            f3_sb = fpool.tile([128, NTB * F3], FP32)
            f_sbs = [f1_sb, f2_sb, f3_sb]

            bounce = dram.tile([SR, 128], BF16)
            table = dram.tile([TR, 128], BF16)

            def write_z_block(b, zsc):
                nc.sync.dma_start(
                    bounce[b * 128:(b + 1) * 128, : zsc.shape[-1]], zsc[:])

            for layer in range(3):
                w = widths[layer]
                # ---- prologue: z_shard = dinv * (f_prev @ W), write bounce
                for b in range(NTB):
                    if layer == 0:
                        lhs = xtp.tile([128, 128], FP32, name=f"xt_{layer}_{b}",
                                       tag="xt")
                        nc.sync.dma_start(lhs[:], xt[:, b * 128:(b + 1) * 128])
                        zp = pzp.tile([128, F1], FP32, name=f"zp_{layer}_{b}",
                                        tag="zz")
                        nc.tensor.matmul(zp[:], lhs[:], w1_sb[:],
                                         start=True, stop=True)
                    else:
                        fprev = f_sbs[layer - 1]
                        wp = [None, w2_sb, w3_sb][layer]
                        pw = widths[layer - 1]
                        ftp = ptrans.tile([pw, 128], FP32, name=f"ftp_{layer}_{b}",
                                         tag="tp")
                        nc.tensor.transpose(
                            ftp[:], fprev[:, b * pw:(b + 1) * pw], id_sb[:])
                        fts = scr.tile([pw, 128], FP32, name=f"fts_{layer}_{b}",
                                       tag="fts")
                        nc.vector.tensor_copy(fts[:], ftp[:])
                        zp = pzp.tile([128, w], FP32, name=f"zp_{layer}_{b}",
                                        tag="zz")
                        nc.tensor.matmul(zp[:], fts[:], wp[:],
                                         start=True, stop=True)
                    zsc = scr.tile([128, w], BF16, name=f"zsc_{layer}_{b}",
                                   tag="zsc")
                    nc.vector.tensor_scalar_mul(
                        zsc[:], zp[:], dinv_sb[:, b:b + 1])
                    write_z_block(b, zsc)
                    if layer == 0:
                        nc.sync.dma_start(
                            bounce[b * 128:(b + 1) * 128, F1:], zero_sb[:, F1:])
                    else:
                        pw = widths[layer - 1]
                        nc.sync.dma_start(
                            bounce[b * 128:(b + 1) * 128, w:pw],
                            zero_sb[:, w:pw])
                if layer == 0:
                    nc.sync.dma_start(bounce[SH:SR, :], zero_sb[:1, :])

                nc.gpsimd.collective_compute(
                    "AllGather", mybir.AluOpType.bypass,
                    replica_groups=[list(range(NCORE))],
                    ins=[bounce.opt()], outs=[table.opt()],
                )

                # ---- gather + one-hot scatter
                tj = 0  # global tile index
                open_block = -1
                for (ch, s_off, n) in calls:
                    slab = spool.tile([128, (NI // 128) * 128], BF16,
                                      name=f"slab_{layer}_{s_off}", tag="slab")
                    idxs = idxp.tile([128, NI // 16], I16,
                                     name=f"idx_{layer}_{s_off}", tag="idx")
                    nc.sync.dma_start(
                        idxs[:, : n // 16],
                        gidx[:, s_off // 16:(s_off + n) // 16])
                    nc.gpsimd.dma_gather(
                        slab[:, : (n // 128) * 128].rearrange(
                            "p (t e) -> p t e", e=128),
                        table[ch * CSR:(ch + 1) * CSR, :],
                        idxs[:, : n // 16],
                        n, n, 128, single_packet=False,
                    )
                    stt = stpool.tile([128, NI], BF16,
                                      name=f"st_{layer}_{s_off}", tag="stt")
                    nc.sync.dma_start(
                        stt[:, : n], st[:, tj * 128: tj * 128 + n])
                    for k in range(n // 128):
                        b = int(tile_tb[tj])
                        first = (tj == 0 or tile_tb[tj - 1] != b
                                 or tile_chunk[tj - 1] != tile_chunk[tj])
                        last = (tj == n_tiles - 1 or tile_tb[tj + 1] != b
                                or tile_chunk[tj + 1] != tile_chunk[tj])
                        if first:
                            pt = psum.tile([128, w], FP32,
                                           name=f"pt_{layer}_{tj}", tag="pt")
                            open_block = b
                        nc.tensor.matmul(
                            pt[:],
                            stt[:, k * 128:(k + 1) * 128],
                            slab[:, k * 128: k * 128 + w],
                            start=first, stop=last,
                        )
                        if last:
                            chk = int(tile_chunk[tj])
                            if chk == 0:
                                nc.vector.tensor_copy(
                                    acc[:, b * w:(b + 1) * w], pt[:])
                            else:
                                nc.vector.tensor_tensor(
                                    acc[:, b * w:(b + 1) * w],
                                    acc[:, b * w:(b + 1) * w], pt[:],
                                    mybir.AluOpType.add)
                        tj += 1

                # ---- epilogue: f = relu(dinv*acc + bias)
                bias = [b1_sb, b2_sb, b3_sb][layer]
                fdst = f_sbs[layer]
                for b in range(NTB):
                    t1 = scr.tile([128, w], FP32, name=f"ep_{layer}_{b}",
                                  tag="ep")
                    nc.vector.tensor_scalar_mul(
                        t1[:], acc[:, b * w:(b + 1) * w], dinv_sb[:, b:b + 1])
                    nc.vector.tensor_tensor(
                        t1[:], t1[:], bias[:, :w], mybir.AluOpType.add)
                    nc.vector.tensor_scalar_max(
                        fdst[:, b * w:(b + 1) * w], t1[:], 0.0)

            # ---- final dense head
            for b in range(NTB):
                pf = pzp.tile([128, FC], FP32, name=f"pf_{b}", tag="zz")
                for li, (fsb, wsb, fw) in enumerate(
                    zip(f_sbs, [wf1_sb, wf2_sb, wf3_sb], widths)
                ):
                    ftp = ptrans.tile([fw, 128], FP32, name=f"fct_{b}_{li}",
                                     tag="tp")
                    nc.tensor.transpose(
                        ftp[:], fsb[:, b * fw:(b + 1) * fw], id_sb[:])
                    fts = scr.tile([fw, 128], FP32, name=f"fcs_{b}_{li}",
                                   tag="fcs")
                    nc.vector.tensor_copy(fts[:], ftp[:])
                    nc.tensor.matmul(pf[:], fts[:], wsb[:],
                                     start=(li == 0), stop=(li == 2))
                t1 = scr.tile([128, FC], FP32, name=f"fo_{b}", tag="fo")
                nc.vector.tensor_tensor(
                    t1[:], pf[:], bf_sb[:], mybir.AluOpType.add)
                t2 = scr.tile([128, FC], FP32, name=f"fo2_{b}", tag="fo2")
                nc.vector.tensor_scalar_max(t2[:], t1[:], 0.0)
                nc.sync.dma_start(out[b * 128:(b + 1) * 128, :], t2[:])

    nc.finalize()
    return nc


def kernel(edges, features, W1, b1, W2, b2, W3, b3, Wfc, bfc):
    global LAST_EXEC_NS
    edges = np.asarray(edges)
    x = np.asarray(features, dtype=np.float32)
    meta, gidx_cores, st_cores, dinv_pad = _host_prep(edges)

    xpad = np.zeros((NV, F0), np.float32)
    xpad[:N] = x
    w1 = np.asarray(W1, np.float32)
    w2 = np.asarray(W2, np.float32)
    w3 = np.asarray(W3, np.float32)
    wfc = np.asarray(Wfc, np.float32)
    bb1 = np.tile(np.asarray(b1, np.float32)[None, :], (128, 1))
    bb2 = np.tile(np.asarray(b2, np.float32)[None, :], (128, 1))
    bb3 = np.tile(np.asarray(b3, np.float32)[None, :], (128, 1))
    bbf = np.tile(np.asarray(bfc, np.float32)[None, :], (128, 1))
    ident = np.eye(128, dtype=np.float32)

    nc = _build(meta)
    in_maps = []
    for c in range(NCORE):
        dv = dinv_pad[c * SH:(c + 1) * SH].reshape(NTB, 128).T.copy()
        in_maps.append({
            "xt": xpad[c * SH:(c + 1) * SH].T.copy(),
            "gidx": gidx_cores[c],
            "st": np.ascontiguousarray(st_cores[c]),
            "dinv_t": dv,
            "w1": w1, "w2": w2, "w3": w3,
            "wf1": wfc[:F1], "wf2": wfc[F1:F1 + F2], "wf3": wfc[F1 + F2:],
            "b1": bb1, "b2": bb2, "b3": bb3, "bf": bbf,
            "ident": ident,
        })

    trace = os.environ.get("GCN_TRACE", "") == "1"
    res = bass_utils.run_bass_kernel_spmd(
        nc, in_maps, list(range(NCORE)), trace=trace)
    LAST_EXEC_NS = res.exec_time_ns
    outs = [res.results[c]["out"] for c in range(NCORE)]
    full = np.concatenate(outs, axis=0)[:N]
    return full.astype(np.float32)


# revision 7
# speedup vs baseline: 1.4675x; 1.4675x over previous
"""GCN message-passing kernel for trn2, 8-core SPMD.

Sharding: nodes (targets) split into 8 shards of 12544 (100352 padded).
Edges partitioned by target shard. Per layer: transformed+scaled node
features (z = dinv * (x @ W)) are AllGathered into a bf16 table
(128-wide rows = 256B, dma_gather's granularity); each core gathers its
edges' source rows in 4 source-chunk passes (int16 index reach), then
scatter-reduces onto its 12544 targets with one-hot matmuls on the PE
(S tiles precomputed host-side), accumulating in PSUM + an SBUF
accumulator. Final Dense head fused on-device.
"""

import os
import numpy as np
import ml_dtypes

import concourse.bacc as bacc
import concourse.mybir as mybir
import concourse.tile as tile
from concourse import bass_utils

N = 100000
NV = 100352          # padded node count (8 * 12544)
SH = 12544           # targets per core
SR = 12545           # allgather shard rows (+1 zero row)
TR = 8 * SR          # table rows
NTB = SH // 128      # 98 target blocks per core
NCHUNK = 4           # source chunks (int16 reach)
CS = 2 * SH          # nodes per chunk
CSR = 2 * SR         # table rows per chunk
NCORE = 8
NI = 8192            # gather indices per call (desc ring limit)
F0, F1, F2, F3, FC = 128, 64, 32, 16, 16

FP32 = mybir.dt.float32
BF16 = mybir.dt.bfloat16
I16 = mybir.dt.int16

LAST_EXEC_NS = None


def _host_prep(edges):
    """Build per-core slot layout, gather indices, one-hot S tiles."""
    e0 = np.asarray(edges[0], dtype=np.int64)
    e1 = np.asarray(edges[1], dtype=np.int64)
    row = np.concatenate([e0, np.arange(N, dtype=np.int64)])
    col = np.concatenate([e1, np.arange(N, dtype=np.int64)])
    deg = np.bincount(col, minlength=N).astype(np.float64)
    dinv = (1.0 / np.sqrt(np.maximum(deg, 1.0))).astype(np.float32)
    dinv_pad = np.ones(NV, np.float32)
    dinv_pad[:N] = dinv

    core = col // SH
    per_core = []
    counts = np.zeros((NCORE, NCHUNK, NTB), np.int64)
    for c in range(NCORE):
        m = core == c
        r, t = row[m], col[m] - SH * c
        ch = r // CS
        tb = t // 128
        order = np.lexsort((t, tb, ch))
        r, t, ch, tb = r[order], t[order], ch[order], tb[order]
        key = ch * NTB + tb
        counts[c] += np.bincount(key, minlength=NCHUNK * NTB).reshape(
            NCHUNK, NTB
        )
        per_core.append((r, t, key))

    padded = (np.ceil(counts.max(axis=0) / 128.0) * 128).astype(np.int64)
    group_base = np.concatenate([[0], np.cumsum(padded.reshape(-1))[:-1]])
    s_tot = int(padded.sum())
    n_tiles = s_tot // 128
    chunk_slots = padded.sum(axis=1)          # slots per chunk
    chunk_base = np.concatenate([[0], np.cumsum(chunk_slots)[:-1]])

    # tile -> target block map (same for all cores)
    tile_tb = np.repeat(
        np.tile(np.arange(NTB), NCHUNK), (padded.reshape(-1) // 128)
    )
    tile_chunk = np.repeat(np.arange(NCHUNK), (padded.sum(axis=1) // 128))

    gidx_cores, st_cores = [], []
    for c in range(NCORE):
        r, t, key = per_core[c]
        sizes = np.bincount(key, minlength=NCHUNK * NTB)
        first = np.concatenate([[0], np.cumsum(sizes)[:-1]])
        rank = np.arange(len(r)) - np.repeat(first, sizes)
        pos = group_base[key] + rank
        shard = r // SH
        loc = r % SH
        idx_in_chunk = (shard % 2) * SR + loc

        slot_idx = np.zeros(s_tot, np.int16)
        slot_idx[pos] = idx_in_chunk.astype(np.int16)
        slot_col = np.full(s_tot, -1, np.int32)
        slot_col[pos] = (t % 128).astype(np.int32)

        # wrapped [16, s_tot/16] -> replicate to 128 partitions
        w = slot_idx.reshape(-1, 16).T
        gidx_cores.append(np.tile(w, (8, 1)).astype(np.int16))

        st = np.zeros((128, n_tiles * 128), np.uint16)
        i = np.flatnonzero(slot_col >= 0)
        p = i % 128
        j = i // 128
        st[p, j * 128 + slot_col[i]] = 0x3F80  # bf16 1.0
        st_cores.append(st.view(ml_dtypes.bfloat16))

    # gather calls: per chunk, slices of <= NI slots
    calls = []
    for ch in range(NCHUNK):
        base, tot = int(chunk_base[ch]), int(chunk_slots[ch])
        off = 0
        while off < tot:
            n = min(NI, tot - off)
            calls.append((ch, base + off, n))
            off += n

    meta = dict(
        n_tiles=n_tiles, tile_tb=tile_tb, tile_chunk=tile_chunk,
        calls=calls, s_tot=s_tot,
    )
    return meta, gidx_cores, st_cores, dinv_pad


def _build(meta):
    nc = bacc.Bacc(None, num_swdge_queues=4)
    xt = nc.declare_dram_parameter("xt", [128, SH], FP32, isOutput=False)
    gidx = nc.declare_dram_parameter(
        "gidx", [128, meta["s_tot"] // 16], I16, isOutput=False)
    st = nc.declare_dram_parameter(
        "st", [128, meta["n_tiles"] * 128], BF16, isOutput=False)
    dinv_t = nc.declare_dram_parameter("dinv_t", [128, NTB], FP32, isOutput=False)
    w1 = nc.declare_dram_parameter("w1", [128, F1], FP32, isOutput=False)
    w2 = nc.declare_dram_parameter("w2", [F1, F2], FP32, isOutput=False)
    w3 = nc.declare_dram_parameter("w3", [F2, F3], FP32, isOutput=False)
    wf1 = nc.declare_dram_parameter("wf1", [F1, FC], FP32, isOutput=False)
    wf2 = nc.declare_dram_parameter("wf2", [F2, FC], FP32, isOutput=False)
    wf3 = nc.declare_dram_parameter("wf3", [F3, FC], FP32, isOutput=False)
    b1 = nc.declare_dram_parameter("b1", [128, F1], FP32, isOutput=False)
    b2 = nc.declare_dram_parameter("b2", [128, F2], FP32, isOutput=False)
    b3 = nc.declare_dram_parameter("b3", [128, F3], FP32, isOutput=False)
    bf = nc.declare_dram_parameter("bf", [128, FC], FP32, isOutput=False)
    ident = nc.declare_dram_parameter("ident", [128, 128], FP32, isOutput=False)
    out = nc.declare_dram_parameter("out", [SH, FC], FP32, isOutput=True)

    tile_tb, tile_chunk = meta["tile_tb"], meta["tile_chunk"]
    calls, n_tiles = meta["calls"], meta["n_tiles"]
    widths = [F1, F2, F3]

    with tile.TileContext(nc) as tc:
        with (
            tc.tile_pool(name="const", bufs=1) as cpool,
            tc.tile_pool(name="slab", bufs=3) as spool,
            tc.tile_pool(name="stp", bufs=2) as stpool,
            tc.tile_pool(name="accp", bufs=1) as apool,
            tc.tile_pool(name="fpool", bufs=1) as fpool,
            tc.tile_pool(name="scr", bufs=2) as scr,
            tc.tile_pool(name="xtp", bufs=2) as xtp,
            tc.tile_pool(name="idxp", bufs=3) as idxp,
            tc.tile_pool(name="ps", bufs=4, space="PSUM") as psum,
            tc.tile_pool(name="ptr", bufs=2, space="PSUM") as ptrans,
            tc.tile_pool(name="pz", bufs=2, space="PSUM") as pzp,
            tc.tile_pool(name="dram", bufs=1, space="DRAM") as dram,
        ):
            dinv_sb = cpool.tile([128, NTB], FP32)
            nc.sync.dma_start(dinv_sb[:], dinv_t[:])
            w1_sb = cpool.tile([128, F1], FP32)
            nc.sync.dma_start(w1_sb[:], w1[:])
            w2_sb = cpool.tile([F1, F2], FP32)
            nc.sync.dma_start(w2_sb[:], w2[:])
            w3_sb = cpool.tile([F2, F3], FP32)
            nc.sync.dma_start(w3_sb[:], w3[:])
            wf1_sb = cpool.tile([F1, FC], FP32)
            nc.sync.dma_start(wf1_sb[:], wf1[:])
            wf2_sb = cpool.tile([F2, FC], FP32)
            nc.sync.dma_start(wf2_sb[:], wf2[:])
            wf3_sb = cpool.tile([F3, FC], FP32)
            nc.sync.dma_start(wf3_sb[:], wf3[:])
            b1_sb = cpool.tile([128, F1], FP32)
            nc.sync.dma_start(b1_sb[:], b1[:])
            b2_sb = cpool.tile([128, F2], FP32)
            nc.sync.dma_start(b2_sb[:], b2[:])
            b3_sb = cpool.tile([128, F3], FP32)
            nc.sync.dma_start(b3_sb[:], b3[:])
            bf_sb = cpool.tile([128, FC], FP32)
            nc.sync.dma_start(bf_sb[:], bf[:])
            id_sb = cpool.tile([128, 128], FP32)
            nc.sync.dma_start(id_sb[:], ident[:])
            zero_sb = cpool.tile([128, 128], BF16)
            nc.gpsimd.memset(zero_sb[:], 0.0)

            acc = apool.tile([128, NTB * F1], FP32)
            f1_sb = fpool.tile([128, NTB * F1], FP32)
            f2_sb = fpool.tile([128, NTB * F2], FP32)
            f3_sb = fpool.tile([128, NTB * F3], FP32)
            f_sbs = [f1_sb, f2_sb, f3_sb]

            bounce = dram.tile([SR, 128], BF16)
            table = dram.tile([TR, 128], BF16)

            def write_z_block(b, zsc):
                nc.sync.dma_start(
                    bounce[b * 128:(b + 1) * 128, : zsc.shape[-1]], zsc[:])

            for layer in range(3):
                w = widths[layer]
                # ---- prologue: z_shard = dinv * (f_prev @ W), write bounce
                for b in range(NTB):
                    if layer == 0:
                        lhs = xtp.tile([128, 128], FP32, name=f"xt_{layer}_{b}",
                                       tag="xt")
                        nc.sync.dma_start(lhs[:], xt[:, b * 128:(b + 1) * 128])
                        zp = pzp.tile([128, F1], FP32, name=f"zp_{layer}_{b}",
                                        tag="zz")
                        nc.tensor.matmul(zp[:], lhs[:], w1_sb[:],
                                         start=True, stop=True)
                    else:
                        fprev = f_sbs[layer - 1]
                        wp = [None, w2_sb, w3_sb][layer]
                        pw = widths[layer - 1]
                        ftp = ptrans.tile([pw, 128], FP32, name=f"ftp_{layer}_{b}",
                                         tag="tp")
                        nc.tensor.transpose(
                            ftp[:], fprev[:, b * pw:(b + 1) * pw], id_sb[:])
                        fts = scr.tile([pw, 128], FP32, name=f"fts_{layer}_{b}",
                                       tag="fts")
                        nc.vector.tensor_copy(fts[:], ftp[:])
                        zp = pzp.tile([128, w], FP32, name=f"zp_{layer}_{b}",
                                        tag="zz")
                        nc.tensor.matmul(zp[:], fts[:], wp[:],
                                         start=True, stop=True)
                    zsc = scr.tile([128, w], BF16, name=f"zsc_{layer}_{b}",
                                   tag="zsc")
                    nc.vector.tensor_scalar_mul(
                        zsc[:], zp[:], dinv_sb[:, b:b + 1])
                    write_z_block(b, zsc)
                    if layer == 0:
                        nc.sync.dma_start(
                            bounce[b * 128:(b + 1) * 128, F1:], zero_sb[:, F1:])
                    else:
                        pw = widths[layer - 1]
                        nc.sync.dma_start(
                            bounce[b * 128:(b + 1) * 128, w:pw],
                            zero_sb[:, w:pw])
                if layer == 0:
                    nc.sync.dma_start(bounce[SH:SR, :], zero_sb[:1, :])

                nc.gpsimd.collective_compute(
                    "AllGather", mybir.AluOpType.bypass,
                    replica_groups=[list(range(NCORE))],
                    ins=[bounce.opt()], outs=[table.opt()],
                )

                # ---- gather + one-hot scatter
                tj = 0  # global tile index
                open_block = -1
                for ci, (ch, s_off, n) in enumerate(calls):
                    slab = spool.tile([128, (NI // 128) * 128], BF16,
                                      name=f"slab_{layer}_{s_off}", tag="slab")
                    idxs = idxp.tile([128, NI // 16], I16,
                                     name=f"idx_{layer}_{s_off}", tag="idx")
                    nc.sync.dma_start(
                        idxs[:, : n // 16],
                        gidx[:, s_off // 16:(s_off + n) // 16])
                    nc.gpsimd.dma_gather(
                        slab[:, : (n // 128) * 128].rearrange(
                            "p (t e) -> p t e", e=128),
                        table[ch * CSR:(ch + 1) * CSR, :],
                        idxs[:, : n // 16],
                        n, n, 128, single_packet=False,
                        queue_num=ci % 4,
                    )
                    stt = stpool.tile([128, NI], BF16,
                                      name=f"st_{layer}_{s_off}", tag="stt")
                    nc.sync.dma_start(
                        stt[:, : n], st[:, tj * 128: tj * 128 + n])
                    for k in range(n // 128):
                        b = int(tile_tb[tj])
                        first = (tj == 0 or tile_tb[tj - 1] != b
                                 or tile_chunk[tj - 1] != tile_chunk[tj])
                        last = (tj == n_tiles - 1 or tile_tb[tj + 1] != b
                                or tile_chunk[tj + 1] != tile_chunk[tj])
                        if first:
                            pt = psum.tile([128, w], FP32,
                                           name=f"pt_{layer}_{tj}", tag="pt")
                            open_block = b
                        nc.tensor.matmul(
                            pt[:],
                            stt[:, k * 128:(k + 1) * 128],
                            slab[:, k * 128: k * 128 + w],
                            start=first, stop=last,
                        )
                        if last:
                            chk = int(tile_chunk[tj])
                            if chk == 0:
                                nc.vector.tensor_copy(
                                    acc[:, b * w:(b + 1) * w], pt[:])
                            else:
                                nc.vector.tensor_tensor(
                                    acc[:, b * w:(b + 1) * w],
                                    acc[:, b * w:(b + 1) * w], pt[:],
                                    mybir.AluOpType.add)
                        tj += 1

                # ---- epilogue: f = relu(dinv*acc + bias)
                bias = [b1_sb, b2_sb, b3_sb][layer]
                fdst = f_sbs[layer]
                for b in range(NTB):
                    t1 = scr.tile([128, w], FP32, name=f"ep_{layer}_{b}",
                                  tag="ep")
                    nc.vector.tensor_scalar_mul(
                        t1[:], acc[:, b * w:(b + 1) * w], dinv_sb[:, b:b + 1])
                    nc.vector.tensor_tensor(
                        t1[:], t1[:], bias[:, :w], mybir.AluOpType.add)
                    nc.vector.tensor_scalar_max(
                        fdst[:, b * w:(b + 1) * w], t1[:], 0.0)

            # ---- final dense head
            for b in range(NTB):
                pf = pzp.tile([128, FC], FP32, name=f"pf_{b}", tag="zz")
                for li, (fsb, wsb, fw) in enumerate(
                    zip(f_sbs, [wf1_sb, wf2_sb, wf3_sb], widths)
                ):
                    ftp = ptrans.tile([fw, 128], FP32, name=f"fct_{b}_{li}",
                                     tag="tp")
                    nc.tensor.transpose(
                        ftp[:], fsb[:, b * fw:(b + 1) * fw], id_sb[:])
                    fts = scr.tile([fw, 128], FP32, name=f"fcs_{b}_{li}",
                                   tag="fcs")
                    nc.vector.tensor_copy(fts[:], ftp[:])
                    nc.tensor.matmul(pf[:], fts[:], wsb[:],
                                     start=(li == 0), stop=(li == 2))
                t1 = scr.tile([128, FC], FP32, name=f"fo_{b}", tag="fo")
                nc.vector.tensor_tensor(
                    t1[:], pf[:], bf_sb[:], mybir.AluOpType.add)
                t2 = scr.tile([128, FC], FP32, name=f"fo2_{b}", tag="fo2")
                nc.vector.tensor_scalar_max(t2[:], t1[:], 0.0)
                nc.sync.dma_start(out[b * 128:(b + 1) * 128, :], t2[:])

    nc.finalize()
    return nc


def kernel(edges, features, W1, b1, W2, b2, W3, b3, Wfc, bfc):
    global LAST_EXEC_NS
    edges = np.asarray(edges)
    x = np.asarray(features, dtype=np.float32)
    meta, gidx_cores, st_cores, dinv_pad = _host_prep(edges)

    xpad = np.zeros((NV, F0), np.float32)
    xpad[:N] = x
    w1 = np.asarray(W1, np.float32)
    w2 = np.asarray(W2, np.float32)
    w3 = np.asarray(W3, np.float32)
    wfc = np.asarray(Wfc, np.float32)
    bb1 = np.tile(np.asarray(b1, np.float32)[None, :], (128, 1))
    bb2 = np.tile(np.asarray(b2, np.float32)[None, :], (128, 1))
    bb3 = np.tile(np.asarray(b3, np.float32)[None, :], (128, 1))
    bbf = np.tile(np.asarray(bfc, np.float32)[None, :], (128, 1))
    ident = np.eye(128, dtype=np.float32)

    nc = _build(meta)
    in_maps = []
    for c in range(NCORE):
        dv = dinv_pad[c * SH:(c + 1) * SH].reshape(NTB, 128).T.copy()
        in_maps.append({
            "xt": xpad[c * SH:(c + 1) * SH].T.copy(),
            "gidx": gidx_cores[c],
            "st": np.ascontiguousarray(st_cores[c]),
            "dinv_t": dv,
            "w1": w1, "w2": w2, "w3": w3,
            "wf1": wfc[:F1], "wf2": wfc[F1:F1 + F2], "wf3": wfc[F1 + F2:],
            "b1": bb1, "b2": bb2, "b3": bb3, "bf": bbf,
            "ident": ident,
        })

    trace = os.environ.get("GCN_TRACE", "") == "1"
    res = bass_utils.run_bass_kernel_spmd(
        nc, in_maps, list(range(NCORE)), trace=trace)
    LAST_EXEC_NS = res.exec_time_ns
    outs = [res.results[c]["out"] for c in range(NCORE)]
    full = np.concatenate(outs, axis=0)[:N]
    return full.astype(np.float32)


# revision 8
# speedup vs baseline: 1.5309x; 1.0432x over previous
"""GCN message-passing kernel for trn2, 8-core SPMD.

Sharding: nodes (targets) split into 8 shards of 12544 (100352 padded).
Edges partitioned by target shard. Per layer: transformed+scaled node
features (z = dinv * (x @ W)) are AllGathered into a bf16 table
(128-wide rows = 256B, dma_gather's granularity); each core gathers its
edges' source rows in 4 source-chunk passes (int16 index reach), then
scatter-reduces onto its 12544 targets with one-hot matmuls on the PE
(S tiles precomputed host-side), accumulating in PSUM + an SBUF
accumulator. Final Dense head fused on-device.
"""

import os
import numpy as np
import ml_dtypes

import concourse.bacc as bacc
import concourse.mybir as mybir
import concourse.tile as tile
from concourse import bass_utils

N = 100000
NV = 100352          # padded node count (8 * 12544)
SH = 12544           # targets per core
SR = 12545           # allgather shard rows (+1 zero row)
TR = 8 * SR          # table rows
NTB = SH // 128      # 98 target blocks per core
NCHUNK = 4           # source chunks (int16 reach)
CS = 2 * SH          # nodes per chunk
CSR = 2 * SR         # table rows per chunk
NCORE = 8
NI = 8192            # gather indices per call (desc ring limit)
F0, F1, F2, F3, FC = 128, 64, 32, 16, 16

FP32 = mybir.dt.float32
BF16 = mybir.dt.bfloat16
I16 = mybir.dt.int16

LAST_EXEC_NS = None


def _host_prep(edges):
    """Build per-core slot layout, gather indices, one-hot S tiles."""
    e0 = np.asarray(edges[0], dtype=np.int64)
    e1 = np.asarray(edges[1], dtype=np.int64)
    row = np.concatenate([e0, np.arange(N, dtype=np.int64)])
    col = np.concatenate([e1, np.arange(N, dtype=np.int64)])
    deg = np.bincount(col, minlength=N).astype(np.float64)
    dinv = (1.0 / np.sqrt(np.maximum(deg, 1.0))).astype(np.float32)
    dinv_pad = np.ones(NV, np.float32)
    dinv_pad[:N] = dinv

    core = col // SH
    per_core = []
    counts = np.zeros((NCORE, NCHUNK, NTB), np.int64)
    for c in range(NCORE):
        m = core == c
        r, t = row[m], col[m] - SH * c
        ch = r // CS
        tb = t // 128
        order = np.lexsort((t, tb, ch))
        r, t, ch, tb = r[order], t[order], ch[order], tb[order]
        key = ch * NTB + tb
        counts[c] += np.bincount(key, minlength=NCHUNK * NTB).reshape(
            NCHUNK, NTB
        )
        per_core.append((r, t, key))

    padded = (np.ceil(counts.max(axis=0) / 128.0) * 128).astype(np.int64)
    group_base = np.concatenate([[0], np.cumsum(padded.reshape(-1))[:-1]])
    s_tot = int(padded.sum())
    n_tiles = s_tot // 128
    chunk_slots = padded.sum(axis=1)          # slots per chunk
    chunk_base = np.concatenate([[0], np.cumsum(chunk_slots)[:-1]])

    # tile -> target block map (same for all cores)
    tile_tb = np.repeat(
        np.tile(np.arange(NTB), NCHUNK), (padded.reshape(-1) // 128)
    )
    tile_chunk = np.repeat(np.arange(NCHUNK), (padded.sum(axis=1) // 128))

    gidx_cores, st_cores = [], []
    for c in range(NCORE):
        r, t, key = per_core[c]
        sizes = np.bincount(key, minlength=NCHUNK * NTB)
        first = np.concatenate([[0], np.cumsum(sizes)[:-1]])
        rank = np.arange(len(r)) - np.repeat(first, sizes)
        pos = group_base[key] + rank
        shard = r // SH
        loc = r % SH
        idx_in_chunk = (shard % 2) * SR + loc

        slot_idx = np.zeros(s_tot, np.int16)
        slot_idx[pos] = idx_in_chunk.astype(np.int16)
        slot_col = np.full(s_tot, -1, np.int32)
        slot_col[pos] = (t % 128).astype(np.int32)

        # wrapped [16, s_tot/16] -> replicate to 128 partitions
        w = slot_idx.reshape(-1, 16).T
        gidx_cores.append(np.tile(w, (8, 1)).astype(np.int16))

        st = np.zeros((128, n_tiles * 128), np.uint16)
        i = np.flatnonzero(slot_col >= 0)
        p = i % 128
        j = i // 128
        st[p, j * 128 + slot_col[i]] = 0x3F80  # bf16 1.0
        st_cores.append(st.view(ml_dtypes.bfloat16))

    # gather calls: per chunk, slices of <= NI slots
    calls = []
    for ch in range(NCHUNK):
        base, tot = int(chunk_base[ch]), int(chunk_slots[ch])
        off = 0
        while off < tot:
            n = min(NI, tot - off)
            calls.append((ch, base + off, n))
            off += n

    meta = dict(
        n_tiles=n_tiles, tile_tb=tile_tb, tile_chunk=tile_chunk,
        calls=calls, s_tot=s_tot,
    )
    return meta, gidx_cores, st_cores, dinv_pad


def _build(meta):
    nc = bacc.Bacc(None, num_swdge_queues=4)
    xt = nc.declare_dram_parameter("xt", [128, SH], FP32, isOutput=False)
    gidx = nc.declare_dram_parameter(
        "gidx", [128, meta["s_tot"] // 16], I16, isOutput=False)
    st = nc.declare_dram_parameter(
        "st", [128, meta["n_tiles"] * 128], BF16, isOutput=False)
    dinv_t = nc.declare_dram_parameter("dinv_t", [128, NTB], FP32, isOutput=False)
    w1 = nc.declare_dram_parameter("w1", [128, F1], FP32, isOutput=False)
    w2 = nc.declare_dram_parameter("w2", [F1, F2], FP32, isOutput=False)
    w3 = nc.declare_dram_parameter("w3", [F2, F3], FP32, isOutput=False)
    wf1 = nc.declare_dram_parameter("wf1", [F1, FC], FP32, isOutput=False)
    wf2 = nc.declare_dram_parameter("wf2", [F2, FC], FP32, isOutput=False)
    wf3 = nc.declare_dram_parameter("wf3", [F3, FC], FP32, isOutput=False)
    b1 = nc.declare_dram_parameter("b1", [128, F1], FP32, isOutput=False)
    b2 = nc.declare_dram_parameter("b2", [128, F2], FP32, isOutput=False)
    b3 = nc.declare_dram_parameter("b3", [128, F3], FP32, isOutput=False)
    bf = nc.declare_dram_parameter("bf", [128, FC], FP32, isOutput=False)
    ident = nc.declare_dram_parameter("ident", [128, 128], FP32, isOutput=False)
    out = nc.declare_dram_parameter("out", [SH, FC], FP32, isOutput=True)

    tile_tb, tile_chunk = meta["tile_tb"], meta["tile_chunk"]
    calls, n_tiles = meta["calls"], meta["n_tiles"]
    widths = [F1, F2, F3]

    with tile.TileContext(nc) as tc:
        with (
            tc.tile_pool(name="const", bufs=1) as cpool,
            tc.tile_pool(name="slab", bufs=4) as spool,
            tc.tile_pool(name="stp", bufs=3) as stpool,
            tc.tile_pool(name="accp", bufs=1) as apool,
            tc.tile_pool(name="fpool", bufs=1) as fpool,
            tc.tile_pool(name="scr", bufs=2) as scr,
            tc.tile_pool(name="xtp", bufs=2) as xtp,
            tc.tile_pool(name="idxp", bufs=4) as idxp,
            tc.tile_pool(name="ps", bufs=4, space="PSUM") as psum,
            tc.tile_pool(name="ptr", bufs=2, space="PSUM") as ptrans,
            tc.tile_pool(name="pz", bufs=2, space="PSUM") as pzp,
            tc.tile_pool(name="dram", bufs=1, space="DRAM") as dram,
        ):
            dinv_sb = cpool.tile([128, NTB], FP32)
            nc.sync.dma_start(dinv_sb[:], dinv_t[:])
            w1_sb = cpool.tile([128, F1], FP32)
            nc.sync.dma_start(w1_sb[:], w1[:])
            w2_sb = cpool.tile([F1, F2], FP32)
            nc.sync.dma_start(w2_sb[:], w2[:])
            w3_sb = cpool.tile([F2, F3], FP32)
            nc.sync.dma_start(w3_sb[:], w3[:])
            wf1_sb = cpool.tile([F1, FC], FP32)
            nc.sync.dma_start(wf1_sb[:], wf1[:])
            wf2_sb = cpool.tile([F2, FC], FP32)
            nc.sync.dma_start(wf2_sb[:], wf2[:])
            wf3_sb = cpool.tile([F3, FC], FP32)
            nc.sync.dma_start(wf3_sb[:], wf3[:])
            b1_sb = cpool.tile([128, F1], FP32)
            nc.sync.dma_start(b1_sb[:], b1[:])
            b2_sb = cpool.tile([128, F2], FP32)
            nc.sync.dma_start(b2_sb[:], b2[:])
            b3_sb = cpool.tile([128, F3], FP32)
            nc.sync.dma_start(b3_sb[:], b3[:])
            bf_sb = cpool.tile([128, FC], FP32)
            nc.sync.dma_start(bf_sb[:], bf[:])
            id_sb = cpool.tile([128, 128], FP32)
            nc.sync.dma_start(id_sb[:], ident[:])
            zero_sb = cpool.tile([128, 128], BF16)
            nc.gpsimd.memset(zero_sb[:], 0.0)

            acc = apool.tile([128, NTB * F1], FP32)
            f1_sb = fpool.tile([128, NTB * F1], FP32)
            f2_sb = fpool.tile([128, NTB * F2], FP32)
            f3_sb = fpool.tile([128, NTB * F3], FP32)
            f_sbs = [f1_sb, f2_sb, f3_sb]

            bounce = dram.tile([SR, 128], BF16)
            table = dram.tile([TR, 128], BF16)

            def write_z_block(b, zsc):
                nc.sync.dma_start(
                    bounce[b * 128:(b + 1) * 128, : zsc.shape[-1]], zsc[:])

            for layer in range(3):
                w = widths[layer]
                # ---- prologue: z_shard = dinv * (f_prev @ W), write bounce
                for b in range(NTB):
                    if layer == 0:
                        lhs = xtp.tile([128, 128], FP32, name=f"xt_{layer}_{b}",
                                       tag="xt")
                        nc.sync.dma_start(lhs[:], xt[:, b * 128:(b + 1) * 128])
                        zp = pzp.tile([128, F1], FP32, name=f"zp_{layer}_{b}",
                                        tag="zz")
                        nc.tensor.matmul(zp[:], lhs[:], w1_sb[:],
                                         start=True, stop=True)
                    else:
                        fprev = f_sbs[layer - 1]
                        wp = [None, w2_sb, w3_sb][layer]
                        pw = widths[layer - 1]
                        ftp = ptrans.tile([pw, 128], FP32, name=f"ftp_{layer}_{b}",
                                         tag="tp")
                        nc.tensor.transpose(
                            ftp[:], fprev[:, b * pw:(b + 1) * pw], id_sb[:])
                        fts = scr.tile([pw, 128], FP32, name=f"fts_{layer}_{b}",
                                       tag="fts")
                        nc.vector.tensor_copy(fts[:], ftp[:])
                        zp = pzp.tile([128, w], FP32, name=f"zp_{layer}_{b}",
                                        tag="zz")
                        nc.tensor.matmul(zp[:], fts[:], wp[:],
                                         start=True, stop=True)
                    zsc = scr.tile([128, w], BF16, name=f"zsc_{layer}_{b}",
                                   tag="zsc")
                    nc.vector.tensor_scalar_mul(
                        zsc[:], zp[:], dinv_sb[:, b:b + 1])
                    write_z_block(b, zsc)
                    if layer == 0:
                        nc.sync.dma_start(
                            bounce[b * 128:(b + 1) * 128, F1:], zero_sb[:, F1:])
                    else:
                        pw = widths[layer - 1]
                        nc.sync.dma_start(
                            bounce[b * 128:(b + 1) * 128, w:pw],
                            zero_sb[:, w:pw])
                if layer == 0:
                    nc.sync.dma_start(bounce[SH:SR, :], zero_sb[:1, :])

                nc.gpsimd.collective_compute(
                    "AllGather", mybir.AluOpType.bypass,
                    replica_groups=[list(range(NCORE))],
                    ins=[bounce.opt()], outs=[table.opt()],
                )

                # ---- gather + one-hot scatter
                tj = 0  # global tile index
                open_block = -1
                for ci, (ch, s_off, n) in enumerate(calls):
                    slab = spool.tile([128, (NI // 128) * 128], BF16,
                                      name=f"slab_{layer}_{s_off}", tag="slab")
                    idxs = idxp.tile([128, NI // 16], I16,
                                     name=f"idx_{layer}_{s_off}", tag="idx")
                    nc.sync.dma_start(
                        idxs[:, : n // 16],
                        gidx[:, s_off // 16:(s_off + n) // 16])
                    nc.gpsimd.dma_gather(
                        slab[:, : (n // 128) * 128].rearrange(
                            "p (t e) -> p t e", e=128),
                        table[ch * CSR:(ch + 1) * CSR, :],
                        idxs[:, : n // 16],
                        n, n, 128, single_packet=False,
                        queue_num=ci % 4,
                    )
                    stt = stpool.tile([128, NI], BF16,
                                      name=f"st_{layer}_{s_off}", tag="stt")
                    nc.sync.dma_start(
                        stt[:, : n], st[:, tj * 128: tj * 128 + n])
                    for k in range(n // 128):
                        b = int(tile_tb[tj])
                        first = (tj == 0 or tile_tb[tj - 1] != b
                                 or tile_chunk[tj - 1] != tile_chunk[tj])
                        last = (tj == n_tiles - 1 or tile_tb[tj + 1] != b
                                or tile_chunk[tj + 1] != tile_chunk[tj])
                        if first:
                            pt = psum.tile([128, w], FP32,
                                           name=f"pt_{layer}_{tj}", tag="pt")
                            open_block = b
                        nc.tensor.matmul(
                            pt[:],
                            stt[:, k * 128:(k + 1) * 128],
                            slab[:, k * 128: k * 128 + w],
                            start=first, stop=last,
                        )
                        if last:
                            chk = int(tile_chunk[tj])
                            if chk == 0:
                                nc.vector.tensor_copy(
                                    acc[:, b * w:(b + 1) * w], pt[:])
                            else:
                                nc.vector.tensor_tensor(
                                    acc[:, b * w:(b + 1) * w],
                                    acc[:, b * w:(b + 1) * w], pt[:],
                                    mybir.AluOpType.add)
                        tj += 1

                # ---- epilogue: f = relu(dinv*acc + bias)
                bias = [b1_sb, b2_sb, b3_sb][layer]
                fdst = f_sbs[layer]
                for b in range(NTB):
                    t1 = scr.tile([128, w], FP32, name=f"ep_{layer}_{b}",
                                  tag="ep")
                    nc.vector.tensor_scalar_mul(
                        t1[:], acc[:, b * w:(b + 1) * w], dinv_sb[:, b:b + 1])
                    nc.vector.tensor_tensor(
                        t1[:], t1[:], bias[:, :w], mybir.AluOpType.add)
                    nc.vector.tensor_scalar_max(
                        fdst[:, b * w:(b + 1) * w], t1[:], 0.0)

            # ---- final dense head
            for b in range(NTB):
                pf = pzp.tile([128, FC], FP32, name=f"pf_{b}", tag="zz")
                for li, (fsb, wsb, fw) in enumerate(
                    zip(f_sbs, [wf1_sb, wf2_sb, wf3_sb], widths)
                ):
                    ftp = ptrans.tile([fw, 128], FP32, name=f"fct_{b}_{li}",
                                     tag="tp")
                    nc.tensor.transpose(
                        ftp[:], fsb[:, b * fw:(b + 1) * fw], id_sb[:])
                    fts = scr.tile([fw, 128], FP32, name=f"fcs_{b}_{li}",
                                   tag="fcs")
                    nc.vector.tensor_copy(fts[:], ftp[:])
                    nc.tensor.matmul(pf[:], fts[:], wsb[:],
                                     start=(li == 0), stop=(li == 2))
                t1 = scr.tile([128, FC], FP32, name=f"fo_{b}", tag="fo")
                nc.vector.tensor_tensor(
                    t1[:], pf[:], bf_sb[:], mybir.AluOpType.add)
                t2 = scr.tile([128, FC], FP32, name=f"fo2_{b}", tag="fo2")
                nc.vector.tensor_scalar_max(t2[:], t1[:], 0.0)
                nc.sync.dma_start(out[b * 128:(b + 1) * 128, :], t2[:])

    nc.finalize()
    return nc


def kernel(edges, features, W1, b1, W2, b2, W3, b3, Wfc, bfc):
    global LAST_EXEC_NS
    edges = np.asarray(edges)
    x = np.asarray(features, dtype=np.float32)
    meta, gidx_cores, st_cores, dinv_pad = _host_prep(edges)

    xpad = np.zeros((NV, F0), np.float32)
    xpad[:N] = x
    w1 = np.asarray(W1, np.float32)
    w2 = np.asarray(W2, np.float32)
    w3 = np.asarray(W3, np.float32)
    wfc = np.asarray(Wfc, np.float32)
    bb1 = np.tile(np.asarray(b1, np.float32)[None, :], (128, 1))
    bb2 = np.tile(np.asarray(b2, np.float32)[None, :], (128, 1))
    bb3 = np.tile(np.asarray(b3, np.float32)[None, :], (128, 1))
    bbf = np.tile(np.asarray(bfc, np.float32)[None, :], (128, 1))
    ident = np.eye(128, dtype=np.float32)

    nc = _build(meta)
    in_maps = []
    for c in range(NCORE):
        dv = dinv_pad[c * SH:(c + 1) * SH].reshape(NTB, 128).T.copy()
        in_maps.append({
            "xt": xpad[c * SH:(c + 1) * SH].T.copy(),
            "gidx": gidx_cores[c],
            "st": np.ascontiguousarray(st_cores[c]),
            "dinv_t": dv,
            "w1": w1, "w2": w2, "w3": w3,
            "wf1": wfc[:F1], "wf2": wfc[F1:F1 + F2], "wf3": wfc[F1 + F2:],
            "b1": bb1, "b2": bb2, "b3": bb3, "bf": bbf,
            "ident": ident,
        })

    trace = os.environ.get("GCN_TRACE", "") == "1"
    res = bass_utils.run_bass_kernel_spmd(
        nc, in_maps, list(range(NCORE)), trace=trace)
    LAST_EXEC_NS = res.exec_time_ns
    outs = [res.results[c]["out"] for c in range(NCORE)]
    full = np.concatenate(outs, axis=0)[:N]
    return full.astype(np.float32)
